# Initial kernel scaffold
#
"""Causal self-attention Trainium2 Bass kernel.

Problem: B=4, T=2048, D=1024, H=16, head_dim=64.
Sharding: 8 cores = (batch b in 0..3) x (head-group g in 0..1, 8 heads each).
Each core computes a partial projection output for its batch over its 512
model dims; the host sums the two partials per batch (b_proj is fed to the
g==0 core only).

All matmuls run in float32r (TF32-like, full PE rate at N>=512).
"""

import numpy as np

import concourse.bacc as bacc
import concourse.bass as bass
import concourse.mybir as mybir
import concourse.tile as tile
from concourse.bass_utils import run_bass_kernel_spmd
from concourse.masks import make_identity

F32 = mybir.dt.float32
F32R = mybir.dt.float32r
AF = mybir.ActivationFunctionType

B, T, D, H = 4, 2048, 1024, 16
HD = 64              # head dim
HPC = 8              # heads per core
DC = HPC * HD        # 512 model dims per core
SCALE = 1.0 / np.sqrt(HD)

_NC_CACHE = {}


def build_nc(t=T):
    """Build the single-core SPMD program. t = sequence length (for small sims)."""
    nt = t // 128          # 128-row tiles over time
    nq = t // 512          # 512-col chunks over time
    ng = t // 1024         # 1024-col groups over time
    KC = D // 128          # 8 contraction chunks for qkv
    MQK = DC // 128        # 4 feature tiles for each of q,k

    nc = bacc.Bacc("TRN2", target_bir_lowering=False, debug=False)

    x_d = nc.dram_tensor("x", [t, D], F32, kind="ExternalInput")
    wq_d = nc.dram_tensor("wq", [D, DC], F32R, kind="ExternalInput")
    wk_d = nc.dram_tensor("wk", [D, DC], F32R, kind="ExternalInput")
    wv_d = nc.dram_tensor("wv", [D, DC], F32R, kind="ExternalInput")
    bq_d = nc.dram_tensor("bq", [1, DC], F32R, kind="ExternalInput")
    bk_d = nc.dram_tensor("bk", [1, DC], F32R, kind="ExternalInput")
    bv_d = nc.dram_tensor("bv", [1, DC], F32R, kind="ExternalInput")
    wp_d = nc.dram_tensor("wp", [DC, D], F32R, kind="ExternalInput")
    bp_d = nc.dram_tensor("bp", [1, D], F32R, kind="ExternalInput")
    out_d = nc.dram_tensor("out", [t, D], F32, kind="ExternalOutput")

    # qk^T bounce: rows 0..DC-1 = q^T features, rows DC..2DC-1 = k^T features
    qkT_d = nc.dram_tensor("qkT_scratch", [2 * DC, t], F32R)

    with tile.TileContext(nc) as tc:
        with tc.tile_pool(name="persist", bufs=1) as persist, \
             tc.tile_pool(name="vpool", bufs=1) as vpool:

            ident = persist.tile([128, 128], F32)
            make_identity(nc, ident[:])
            ones = persist.tile([1, 512], F32R)
            nc.vector.memset(ones[:], 1.0)
            bq_s = persist.tile([1, DC], F32R)
            bk_s = persist.tile([1, DC], F32R)
            bv_s = persist.tile([1, DC], F32R)
            bp_s = persist.tile([1, D], F32R)
            nc.sync.dma_start(bq_s[:], bq_d[:])
            nc.sync.dma_start(bk_s[:], bk_d[:])
            nc.sync.dma_start(bv_s[:], bv_d[:])
            nc.sync.dma_start(bp_s[:], bp_d[:])

            # v' tiles: per time-tile, [128, 8*65]; col h*65+64 holds ones
            vp = [vpool.tile([128, HPC * (HD + 1)], F32R, name=f"vp{i}", tag="vp")
                  for i in range(nt)]

            # ---------------- Phase A: x^T, qkv ----------------
            with tc.tile_pool(name="phA_sb", bufs=1) as pa, \
                 tc.tile_pool(name="phA_stream", bufs=4) as pstr, \
                 tc.tile_pool(name="phA_evac", bufs=3) as pev, \
                 tc.tile_pool(name="phA_ps", bufs=4, space="PSUM") as pps:

                # x^T resident: KC tiles [128, t]
                xT = [pa.tile([128, t], F32R, name=f"xT{k}", tag="xT")
                      for k in range(KC)]

                # w tiles resident per kc: [128, DC]
                wq_s = [pa.tile([128, DC], F32R, name=f"wq{k}", tag="wq") for k in range(KC)]
                wk_s = [pa.tile([128, DC], F32R, name=f"wk{k}", tag="wk") for k in range(KC)]
                wv_s = [pa.tile([128, DC], F32R, name=f"wv{k}", tag="wv") for k in range(KC)]
                for k in range(KC):
                    nc.sync.dma_start(wq_s[k][:], wq_d[k * 128:(k + 1) * 128, :])
                    nc.sync.dma_start(wk_s[k][:], wk_d[k * 128:(k + 1) * 128, :])
                    nc.sync.dma_start(wv_s[k][:], wv_d[k * 128:(k + 1) * 128, :])

                for i in range(nt):
                    nc.vector.memset(vp[i][:], 1.0)

                # transpose x into xT
                for tg in range(nt // 4):
                    xn = [pstr.tile([128, D], F32, name=f"xn{tg}_{j}", tag="xn")
                          for j in range(4)]
                    for j in range(4):
                        tt = tg * 4 + j
                        nc.sync.dma_start(xn[j][:], x_d[tt * 128:(tt + 1) * 128, :])
                    for k in range(KC):
                        ps = pps.tile([128, 512], F32, name="trps", tag="ps")
                        for j in range(4):
                            nc.tensor.transpose(
                                ps[:, j * 128:(j + 1) * 128],
                                xn[j][:, k * 128:(k + 1) * 128],
                                ident[:],
                            )
                        nc.vector.tensor_copy(
                            xT[k][:, tg * 512:(tg + 1) * 512], ps[:])

                # q^T / k^T -> psum -> sbuf -> DRAM bounce
                for sec, (w_s, b_s) in enumerate(((wq_s, bq_s), (wk_s, bk_s))):
                    for m in range(MQK):
                        for n in range(nq):
                            ps = pps.tile([128, 512], F32, name="qkps", tag="ps")
                            for k in range(KC):
                                nc.tensor.matmul(
                                    ps[:],
                                    w_s[k][:, m * 128:(m + 1) * 128],
                                    xT[k][:, n * 512:(n + 1) * 512],
                                    start=(k == 0), stop=False)
                            nc.tensor.matmul(
                                ps[:],
                                b_s[0:1, m * 128:(m + 1) * 128],
                                ones[0:1, :],
                                start=False, stop=True)
                            ev = pev.tile([128, 512], F32R, name="qkev", tag="ev")
                            nc.vector.tensor_copy(ev[:], ps[:])
                            nc.sync.dma_start(
                                qkT_d[sec * DC + m * 128: sec * DC + (m + 1) * 128,
                                      n * 512:(n + 1) * 512],
                                ev[:])

                # v natural (+bias) -> strided copy into v' tiles
                for tt in range(nt):
                    ps = pps.tile([128, 512], F32, name="vps", tag="ps")
                    for k in range(KC):
                        nc.tensor.matmul(
                            ps[:],
                            xT[k][:, tt * 128:(tt + 1) * 128],
                            wv_s[k][:],
                            start=(k == 0), stop=False)
                    nc.tensor.matmul(
                        ps[:], ones[0:1, 0:128], bv_s[0:1, :],
                        start=False, stop=True)
                    nc.vector.tensor_copy(
                        vp[tt].rearrange("p (h e) -> p h e", e=HD + 1)[:, :, 0:HD],
                        ps.rearrange("p (h e) -> p h e", e=HD))

            # ---------------- Phase B: attention ----------------
            with tc.tile_pool(name="yT", bufs=1) as ypool:
                yT = [ypool.tile([128, t], F32R, name=f"yT{f}", tag="yT")
                      for f in range(MQK)]

                with tc.tile_pool(name="qk_pair", bufs=4) as pqk, \
                     tc.tile_pool(name="esb", bufs=4) as pesb, \
                     tc.tile_pool(name="norm", bufs=4) as pnorm, \
                     tc.tile_pool(name="sc_ps", bufs=2, space="PSUM") as pscps, \
                     tc.tile_pool(name="y_ps", bufs=4, space="PSUM") as pyps:

                    for f in range(MQK):
                        qf = pqk.tile([128, t], F32R, name=f"qf{f}", tag="qf")
                        kf = pqk.tile([128, t], F32R, name=f"kf{f}", tag="kf")
                        nc.sync.dma_start(qf[:], qkT_d[f * 128:(f + 1) * 128, :])
                        nc.sync.dma_start(kf[:], qkT_d[DC + f * 128:DC + (f + 1) * 128, :])

                        for hh in range(2):
                            h = 2 * f + hh
                            qh = qf[hh * HD:(hh + 1) * HD, :]
                            kh = kf[hh * HD:(hh + 1) * HD, :]
                            y_acc = [pyps.tile([HD + 1, 512], F32,
                                               name=f"yacc{h}_{n}", tag="yacc")
                                     for n in range(nq)]
                            for kc in range(nt):
                                jdiag = kc // 8
                                nmin = kc // 4
                                for j in range(jdiag, ng):
                                    ns = [n for n in (2 * j, 2 * j + 1)
                                          if n >= nmin and n < nq]
                                    if not ns:
                                        continue
                                    sp = pscps.tile([128, 1024], F32,
                                                    name="scps", tag="scps")
                                    for n in ns:
                                        nc.tensor.matmul(
                                            sp[:, (n - 2 * j) * 512:(n - 2 * j + 1) * 512],
                                            kh[:, kc * 128:(kc + 1) * 128],
                                            qh[:, n * 512:(n + 1) * 512],
                                            start=True, stop=True)
                                    esb = pesb.tile([128, 1024], F32R,
                                                    name="esb", tag="esb")
                                    nc.scalar.activation(esb[:], sp[:], AF.Exp,
                                                         scale=float(SCALE))
                                    if j == jdiag:
                                        w = 128 * (kc % 8) + 128
                                        nc.gpsimd.affine_select(
                                            out=esb[:, 0:w], in_=esb[:, 0:w],
                                            compare_op=mybir.AluOpType.is_ge,
                                            fill=0.0,
                                            base=1024 * j - 128 * kc,
                                            pattern=[[1, w]],
                                            channel_multiplier=-1)
                                    for n in ns:
                                        nc.tensor.matmul(
                                            y_acc[n][:],
                                            vp[kc][:, h * (HD + 1):(h + 1) * (HD + 1)],
                                            esb[:, (n - 2 * j) * 512:(n - 2 * j + 1) * 512],
                                            start=(kc == 0), stop=(kc == 4 * n + 3))
                            # normalize: yT[f][hh*64:, n*512:] = y/denom
                            for n in range(nq):
                                rec = pnorm.tile([1, 512], F32, name="rec", tag="rec")
                                nc.vector.reciprocal(rec[:], y_acc[n][HD:HD + 1, :])
                                rb = pnorm.tile([HD, 512], F32, name="rb", tag="rb")
                                nc.gpsimd.partition_broadcast(rb[:], rec[:])
                                nc.vector.tensor_mul(
                                    yT[f][hh * HD:(hh + 1) * HD,
                                          n * 512:(n + 1) * 512],
                                    y_acc[n][0:HD, :], rb[:])

                # ---------------- Phase C: projection ----------------
                with tc.tile_pool(name="phC_sb", bufs=1) as pc, \
                     tc.tile_pool(name="phC_evac", bufs=3) as pcev, \
                     tc.tile_pool(name="phC_ps", bufs=4, space="PSUM") as pcps:
                    wp_s = [pc.tile([128, D], F32R, name=f"wp{m}", tag="wp")
                            for m in range(MQK)]
                    for m in range(MQK):
                        nc.sync.dma_start(wp_s[m][:], wp_d[m * 128:(m + 1) * 128, :])
                    for qt in range(nt):
                        for oc in range(D // 512):
                            ps = pcps.tile([128, 512], F32, name="prps", tag="prps")
                            for m in range(MQK):
                                nc.tensor.matmul(
                                    ps[:],
                                    yT[m][:, qt * 128:(qt + 1) * 128],
                                    wp_s[m][:, oc * 512:(oc + 1) * 512],
                                    start=(m == 0), stop=False)
                            nc.tensor.matmul(
                                ps[:], ones[0:1, 0:128],
                                bp_s[0:1, oc * 512:(oc + 1) * 512],
                                start=False, stop=True)
                            ev = pcev.tile([128, 512], F32, name="prev", tag="prev")
                            nc.vector.tensor_copy(ev[:], ps[:])
                            nc.sync.dma_start(
                                out_d[qt * 128:(qt + 1) * 128,
                                      oc * 512:(oc + 1) * 512],
                                ev[:])

    nc.finalize()
    return nc


def make_in_maps(x, w_attn, b_attn, w_proj, b_proj):
    x = np.ascontiguousarray(np.asarray(x, dtype=np.float32))
    w_attn = np.asarray(w_attn, dtype=np.float32)
    b_attn = np.asarray(b_attn, dtype=np.float32)
    w_proj = np.asarray(w_proj, dtype=np.float32)
    b_proj = np.asarray(b_proj, dtype=np.float32)
    in_maps = []
    for c in range(8):
        b, g = c // 2, c % 2
        sl = slice(DC * g, DC * (g + 1))
        in_maps.append({
            "x": np.ascontiguousarray(x[b]),
            "wq": np.ascontiguousarray(w_attn[:, 0 * D:][:, sl]),
            "wk": np.ascontiguousarray(w_attn[:, 1 * D:][:, sl]),
            "wv": np.ascontiguousarray(w_attn[:, 2 * D:][:, sl]),
            "bq": np.ascontiguousarray(b_attn[0 * D:1 * D][sl][None, :]),
            "bk": np.ascontiguousarray(b_attn[1 * D:2 * D][sl][None, :]),
            "bv": np.ascontiguousarray(b_attn[2 * D:3 * D][sl][None, :]),
            "wp": np.ascontiguousarray(w_proj[sl, :]),
            "bp": np.ascontiguousarray(
                (b_proj if g == 0 else np.zeros_like(b_proj))[None, :]),
        })
    return in_maps


def kernel(x, w_attn, b_attn, w_proj, b_proj, _trace=False, _trace_kwargs=None):
    if "nc" not in _NC_CACHE:
        _NC_CACHE["nc"] = build_nc()
    nc = _NC_CACHE["nc"]
    in_maps = make_in_maps(x, w_attn, b_attn, w_proj, b_proj)
    kw = {}
    if _trace:
        kw["trace"] = True
        if _trace_kwargs:
            kw.update(_trace_kwargs)
    res = run_bass_kernel_spmd(nc, in_maps, core_ids=list(range(8)), **kw)
    outs = [res.results[c]["out"] for c in range(8)]
    out = np.empty((B, T, D), dtype=np.float32)
    for b in range(B):
        np.add(outs[2 * b], outs[2 * b + 1], out=out[b])
    kernel._last_results = res
    return out


if __name__ == "__main__":
    nc = build_nc()
    print("built ok")


# revision 12
# speedup vs baseline: 1.0930x; 1.0930x over previous
"""Causal self-attention Trainium2 Bass kernel.

Problem: B=4, T=2048, D=1024, H=16, head_dim=64.
Sharding: 8 cores = (batch b in 0..3) x (head-group g in 0..1, 8 heads each).
Each core computes a partial projection output for its batch over its 512
model dims; the host sums the two partials per batch (b_proj is fed to the
g==0 core only).

All matmuls run in float32r (TF32-like, full PE rate at N>=512).
"""

import numpy as np

import concourse.bacc as bacc
import concourse.bass as bass
import concourse.mybir as mybir
import concourse.tile as tile
from concourse.bass_utils import run_bass_kernel_spmd
from concourse.masks import make_identity

F32 = mybir.dt.float32
F32R = mybir.dt.float32r
AF = mybir.ActivationFunctionType

B, T, D, H = 4, 2048, 1024, 16
HD = 64              # head dim
HPC = 8              # heads per core
DC = HPC * HD        # 512 model dims per core
SCALE = 1.0 / np.sqrt(HD)

_NC_CACHE = {}


def build_nc(t=T, reps=1):
    """Build the single-core SPMD program. t = sequence length (for small sims).
    reps>1 repeats the whole computation for device-time measurement."""
    nt = t // 128          # 128-row tiles over time
    nq = t // 512          # 512-col chunks over time
    ng = t // 1024         # 1024-col groups over time
    KC = D // 128          # 8 contraction chunks for qkv
    MQK = DC // 128        # 4 feature tiles for each of q,k

    nc = bacc.Bacc("TRN2", target_bir_lowering=False, debug=False)

    x_d = nc.dram_tensor("x", [t, D], F32, kind="ExternalInput")
    wq_d = nc.dram_tensor("wq", [D, DC], F32R, kind="ExternalInput")
    wk_d = nc.dram_tensor("wk", [D, DC], F32R, kind="ExternalInput")
    wv_d = nc.dram_tensor("wv", [D, DC], F32R, kind="ExternalInput")
    bq_d = nc.dram_tensor("bq", [1, DC], F32R, kind="ExternalInput")
    bk_d = nc.dram_tensor("bk", [1, DC], F32R, kind="ExternalInput")
    bv_d = nc.dram_tensor("bv", [1, DC], F32R, kind="ExternalInput")
    wp_d = nc.dram_tensor("wp", [DC, D], F32R, kind="ExternalInput")
    bp_d = nc.dram_tensor("bp", [1, D], F32R, kind="ExternalInput")
    ones_d = nc.dram_tensor("cones", [1, 512], F32R, kind="ExternalInput")
    out_d = nc.dram_tensor("out", [t, D], F32, kind="ExternalOutput")

    with tile.TileContext(nc) as tc:
      for _rep in range(reps):
        with tc.tile_pool(name="persist", bufs=1) as persist, \
             tc.tile_pool(name="vpool", bufs=1) as vpool, \
             tc.tile_pool(name="dramp", bufs=1, space="DRAM") as dramp:

            # qk^T bounce: rows 0..DC-1 = q^T feats, rows DC..2DC-1 = k^T feats
            qkT_d = dramp.tile([2 * DC, t], F32R)

            ident = persist.tile([128, 128], F32)
            make_identity(nc, ident[:])
            ones = persist.tile([1, 512], F32R)
            nc.sync.dma_start(ones[:], ones_d[:])
            # [128, 8] broadcast of ones for the v' ones-columns
            ones_bc = persist.tile([128, HPC], F32R)
            nc.gpsimd.dma_start(ones_bc[:], ones_d[0:1, 0:HPC].to_broadcast([128, HPC]))
            bq_s = persist.tile([1, DC], F32R)
            bk_s = persist.tile([1, DC], F32R)
            bv_s = persist.tile([1, DC], F32R)
            bp_s = persist.tile([1, D], F32R)
            nc.sync.dma_start(bq_s[:], bq_d[:])
            nc.sync.dma_start(bk_s[:], bk_d[:])
            nc.sync.dma_start(bv_s[:], bv_d[:])
            nc.sync.dma_start(bp_s[:], bp_d[:])

            # v' tiles: per time-tile, [128, 8*65]; col h*65+64 holds ones
            vp = [vpool.tile([128, HPC * (HD + 1)], F32R, name=f"vp{i}", tag=f"vp{i}")
                  for i in range(nt)]

            # ---------------- Phase A: x^T, qkv ----------------
            with tc.tile_pool(name="phA_sb", bufs=1) as pa, \
                 tc.tile_pool(name="phA_stream", bufs=4) as pstr, \
                 tc.tile_pool(name="phA_evac", bufs=3) as pev, \
                 tc.tile_pool(name="phA_ps", bufs=4, space="PSUM") as pps:

                # x^T resident: KC tiles [128, t]
                xT = [pa.tile([128, t], F32R, name=f"xT{k}", tag=f"xT{k}")
                      for k in range(KC)]

                # w tiles resident per kc: [128, DC]
                wq_s = [pa.tile([128, DC], F32R, name=f"wq{k}", tag=f"wq{k}") for k in range(KC)]
                wk_s = [pa.tile([128, DC], F32R, name=f"wk{k}", tag=f"wk{k}") for k in range(KC)]
                wv_s = [pa.tile([128, DC], F32R, name=f"wv{k}", tag=f"wv{k}") for k in range(KC)]
                for k in range(KC):
                    nc.sync.dma_start(wq_s[k][:], wq_d[k * 128:(k + 1) * 128, :])
                    nc.sync.dma_start(wk_s[k][:], wk_d[k * 128:(k + 1) * 128, :])
                    nc.sync.dma_start(wv_s[k][:], wv_d[k * 128:(k + 1) * 128, :])

                # transpose x into xT
                for tg in range(nt // 4):
                    xn = [pstr.tile([128, D], F32, name=f"xn{tg}_{j}", tag="xn")
                          for j in range(4)]
                    for j in range(4):
                        tt = tg * 4 + j
                        nc.sync.dma_start(xn[j][:], x_d[tt * 128:(tt + 1) * 128, :])
                    for k in range(KC):
                        ps = pps.tile([128, 512], F32, name="trps", tag="ps")
                        for j in range(4):
                            nc.tensor.transpose(
                                ps[:, j * 128:(j + 1) * 128],
                                xn[j][:, k * 128:(k + 1) * 128],
                                ident[:],
                            )
                        nc.vector.tensor_copy(
                            xT[k][:, tg * 512:(tg + 1) * 512], ps[:])

                # q^T / k^T -> psum -> sbuf -> DRAM bounce
                for sec, (w_s, b_s) in enumerate(((wq_s, bq_s), (wk_s, bk_s))):
                    for m in range(MQK):
                        for n in range(nq):
                            ps = pps.tile([128, 512], F32, name="qkps", tag="ps")
                            for k in range(KC):
                                nc.tensor.matmul(
                                    ps[:],
                                    w_s[k][:, m * 128:(m + 1) * 128],
                                    xT[k][:, n * 512:(n + 1) * 512],
                                    start=(k == 0), stop=False)
                            nc.tensor.matmul(
                                ps[:],
                                b_s[0:1, m * 128:(m + 1) * 128],
                                ones[0:1, :],
                                start=False, stop=True)
                            ev = pev.tile([128, 512], F32R, name="qkev", tag="ev")
                            nc.vector.tensor_copy(ev[:], ps[:])
                            nc.sync.dma_start(
                                qkT_d[sec * DC + m * 128: sec * DC + (m + 1) * 128,
                                      n * 512:(n + 1) * 512],
                                ev[:])

                # v natural (+bias) -> strided copy into v' tiles
                for tt in range(nt):
                    ps = pps.tile([128, 512], F32, name="vps", tag="ps")
                    for k in range(KC):
                        nc.tensor.matmul(
                            ps[:],
                            xT[k][:, tt * 128:(tt + 1) * 128],
                            wv_s[k][:],
                            start=(k == 0), stop=False)
                    nc.tensor.matmul(
                        ps[:], ones[0:1, 0:128], bv_s[0:1, :],
                        start=False, stop=True)
                    nc.vector.tensor_copy(
                        vp[tt].rearrange("p (h e) -> p h e", e=HD + 1)[:, :, 0:HD],
                        ps.rearrange("p (h e) -> p h e", e=HD))
                    nc.vector.tensor_copy(
                        vp[tt].rearrange("p (h e) -> p h e", e=HD + 1)[:, :, HD:HD + 1],
                        ones_bc[:].unsqueeze(2))

            # ---------------- Phase B: attention ----------------
            with tc.tile_pool(name="yT", bufs=1) as ypool:
                yT = [ypool.tile([128, t], F32R, name=f"yT{f}", tag=f"yT{f}")
                      for f in range(MQK)]

                with tc.tile_pool(name="qk_pair", bufs=2) as pqk, \
                     tc.tile_pool(name="esb", bufs=4) as pesb, \
                     tc.tile_pool(name="norm", bufs=4) as pnorm, \
                     tc.tile_pool(name="sc_ps", bufs=2, space="PSUM") as pscps, \
                     tc.tile_pool(name="y_ps", bufs=4, space="PSUM") as pyps:

                    for f in range(MQK):
                        qf = pqk.tile([128, t], F32R, name=f"qf{f}", tag="qf")
                        kf = pqk.tile([128, t], F32R, name=f"kf{f}", tag="kf")
                        nc.sync.dma_start(qf[:], qkT_d[f * 128:(f + 1) * 128, :])
                        nc.sync.dma_start(kf[:], qkT_d[DC + f * 128:DC + (f + 1) * 128, :])

                        for hh in range(2):
                            h = 2 * f + hh
                            qh = qf[hh * HD:(hh + 1) * HD, :]
                            kh = kf[hh * HD:(hh + 1) * HD, :]
                            y_acc = [pyps.tile([HD + 1, 512], F32,
                                               name=f"yacc{h}_{n}", tag="yacc")
                                     for n in range(nq)]
                            for kc in range(nt):
                                jdiag = kc // 8
                                nmin = kc // 4
                                for j in range(jdiag, ng):
                                    ns = [n for n in (2 * j, 2 * j + 1)
                                          if n >= nmin and n < nq]
                                    if not ns:
                                        continue
                                    sp = pscps.tile([128, 1024], F32,
                                                    name="scps", tag="scps")
                                    lo = (min(ns) - 2 * j) * 512
                                    for n in ns:
                                        nc.tensor.matmul(
                                            sp[:, (n - 2 * j) * 512:(n - 2 * j + 1) * 512],
                                            kh[:, kc * 128:(kc + 1) * 128],
                                            qh[:, n * 512:(n + 1) * 512],
                                            start=True, stop=True)
                                    esb = pesb.tile([128, 1024], F32R,
                                                    name="esb", tag="esb")
                                    nc.scalar.activation(esb[:, lo:], sp[:, lo:],
                                                         AF.Exp,
                                                         scale=float(SCALE))
                                    if j == jdiag:
                                        w = 128 * (kc % 8) + 128
                                        if w > lo:
                                            nc.gpsimd.affine_select(
                                                out=esb[:, lo:w], in_=esb[:, lo:w],
                                                compare_op=mybir.AluOpType.is_ge,
                                                fill=0.0,
                                                base=1024 * j - 128 * kc + lo,
                                                pattern=[[1, w - lo]],
                                                channel_multiplier=-1)
                                    for n in ns:
                                        nc.tensor.matmul(
                                            y_acc[n][:],
                                            vp[kc][:, h * (HD + 1):(h + 1) * (HD + 1)],
                                            esb[:, (n - 2 * j) * 512:(n - 2 * j + 1) * 512],
                                            start=(kc == 0), stop=(kc == 4 * n + 3))
                            # normalize: yT[f][hh*64:, n*512:] = y/denom
                            for n in range(nq):
                                rec = pnorm.tile([1, 512], F32, name="rec", tag="rec")
                                nc.vector.reciprocal(rec[:], y_acc[n][HD:HD + 1, :])
                                rb = pnorm.tile([HD, 512], F32, name="rb", tag="rb")
                                nc.gpsimd.partition_broadcast(rb[:], rec[:])
                                nc.vector.tensor_mul(
                                    yT[f][hh * HD:(hh + 1) * HD,
                                          n * 512:(n + 1) * 512],
                                    y_acc[n][0:HD, :], rb[:])

                # ---------------- Phase C: projection ----------------
                with tc.tile_pool(name="phC_sb", bufs=1) as pc, \
                     tc.tile_pool(name="phC_evac", bufs=3) as pcev, \
                     tc.tile_pool(name="phC_ps", bufs=4, space="PSUM") as pcps:
                    wp_s = [pc.tile([128, D], F32R, name=f"wp{m}", tag=f"wp{m}")
                            for m in range(MQK)]
                    for m in range(MQK):
                        nc.sync.dma_start(wp_s[m][:], wp_d[m * 128:(m + 1) * 128, :])
                    for qt in range(nt):
                        for oc in range(D // 512):
                            ps = pcps.tile([128, 512], F32, name="prps", tag="prps")
                            for m in range(MQK):
                                nc.tensor.matmul(
                                    ps[:],
                                    yT[m][:, qt * 128:(qt + 1) * 128],
                                    wp_s[m][:, oc * 512:(oc + 1) * 512],
                                    start=(m == 0), stop=False)
                            nc.tensor.matmul(
                                ps[:], ones[0:1, 0:128],
                                bp_s[0:1, oc * 512:(oc + 1) * 512],
                                start=False, stop=True)
                            ev = pcev.tile([128, 512], F32, name="prev", tag="prev")
                            nc.vector.tensor_copy(ev[:], ps[:])
                            nc.sync.dma_start(
                                out_d[qt * 128:(qt + 1) * 128,
                                      oc * 512:(oc + 1) * 512],
                                ev[:])

    nc.finalize()
    return nc


def make_in_maps(x, w_attn, b_attn, w_proj, b_proj):
    x = np.ascontiguousarray(np.asarray(x, dtype=np.float32))
    w_attn = np.asarray(w_attn, dtype=np.float32)
    b_attn = np.asarray(b_attn, dtype=np.float32)
    w_proj = np.asarray(w_proj, dtype=np.float32)
    b_proj = np.asarray(b_proj, dtype=np.float32)
    in_maps = []
    for c in range(8):
        b, g = c // 2, c % 2
        sl = slice(DC * g, DC * (g + 1))
        in_maps.append({
            "x": np.ascontiguousarray(x[b]),
            "wq": np.ascontiguousarray(w_attn[:, 0 * D:][:, sl]),
            "wk": np.ascontiguousarray(w_attn[:, 1 * D:][:, sl]),
            "wv": np.ascontiguousarray(w_attn[:, 2 * D:][:, sl]),
            "bq": np.ascontiguousarray(b_attn[0 * D:1 * D][sl][None, :]),
            "bk": np.ascontiguousarray(b_attn[1 * D:2 * D][sl][None, :]),
            "bv": np.ascontiguousarray(b_attn[2 * D:3 * D][sl][None, :]),
            "wp": np.ascontiguousarray(w_proj[sl, :]),
            "bp": np.ascontiguousarray(
                (b_proj if g == 0 else np.zeros_like(b_proj))[None, :]),
            "cones": np.ones((1, 512), dtype=np.float32),
        })
    return in_maps


def kernel(x, w_attn, b_attn, w_proj, b_proj, _trace=False, _trace_kwargs=None):
    if "nc" not in _NC_CACHE:
        _NC_CACHE["nc"] = build_nc()
    nc = _NC_CACHE["nc"]
    in_maps = make_in_maps(x, w_attn, b_attn, w_proj, b_proj)
    kw = {}
    if _trace:
        kw["trace"] = True
        if _trace_kwargs:
            kw.update(_trace_kwargs)
    res = run_bass_kernel_spmd(nc, in_maps, core_ids=list(range(8)), **kw)
    outs = [res.results[c]["out"] for c in range(8)]
    out = np.empty((B, T, D), dtype=np.float32)
    for b in range(B):
        np.add(outs[2 * b], outs[2 * b + 1], out=out[b])
    kernel._last_results = res
    return out


if __name__ == "__main__":
    nc = build_nc()
    print("built ok")
